# revision 85
# baseline (speedup 1.0000x reference)
"""Causal self-attention (B=2, L=2048, E=2048, H=16, HD=128) on 8 trn2 cores.

Sharding: core c = (b, g) with b = c // 4 (batch), g = c % 4 (head group of 4).
Each core computes QKV projection for its 4 heads on its batch, causal
attention with RoPE, and a partial output projection (its heads' slice of
w_proj rows). Host sums the 4 partial projections per batch.

All matmuls run in bf16/fp16 with fp32 PSUM accumulation (measured
end-to-end rel. error ~4.3e-3 vs the fp32 reference).

Key device-side structure (per core):
  - phase 1, per 512-wide l-chunk: q/k/v projections as K-accumulated
    matmuls; rope fused right behind each q/k chunk:
        rot = (q * cs) - half_swap(q * ss)      [2 DVE muls + DMA swap + sub]
    with cs/ss host-prebuilt [128, L] tables (softmax scale folded in); the
    half-swap (rope pair partner lives at p +- 64 because head rows are
    perm'd to even|odd order) is an SBUF->SBUF DMA on the gpsimd queue — no
    PE or cross-partition ALU involvement.
  - phase 2: scores computed transposed (sT[j,i] = k_j . q_i) so P@V needs
    no transpose; softmax without max-subtraction: exp biased by -9 so the
    probabilities fit fp16; the denominator is accumulated across key blocks
    on the DVE into an fp16 SBUF tile S, then ONE ones-matmul per (head,
    l-chunk) reduces S over partitions and broadcasts Z (instead of a
    per-block ones-matmul — 8x fewer PE columns); causal masking by skipping
    upper-triangle blocks + 4 static diagonal masks (applied on gpsimd);
    software-pipelined with a 3-deep score-matmul lookahead, and each job's
    finalize (Z-reduce / reciprocal / y-scale) deferred into the next job's
    steps so the PE never waits on the exp/S-add chain.
  - phase 3: partial out-projection, [f, l] layout, fp16 partials; each
    l-chunk's projection is queued when its 4 heads finish and dribbled out
    one 4-matmul group per TWO attention steps — 64 groups cover all 160
    steps, filling PE idle in exp-bound stretches and spreading the output
    DMA. Jobs run in ic order (0,2,3,1) so projection work exists after just
    16 steps and the final finalize chain belongs to a medium-size job.

Scheduling notes (hard-won):
  - The PE runs ~0.42 ns/col in bf16/fp16 only when kept continuously busy;
    it is essentially at the column-count roofline here, so all remaining
    wins come from removing columns or removing stalls.
  - DVE ops cost ~free_size cycles regardless of partition count; 2-byte
    SBUF-only operands run at 2x. GPSIMD tensor ops are ~3x slower and
    cannot touch PSUM; gpsimd's software DMA queue is the fastest loader.
  - The chip sometimes runs whole executions downclocked (2.4 GHz ->
    ~1.8 GHz, +17% wall; matmul median 378ns vs 453ns in the profile),
    mostly after many back-to-back runs (thermal). Benchmark conclusions
    drawn from a hot chip are garbage — rest the device ~90-150s and check
    the matmul-duration median before trusting a comparison.
  - Startup DMAs are split across the sync/gpsimd/scalar queues in exact PE
    consumption order; attention-phase constants prefetch mid-phase-1.

Device layouts (per core):
  xt    [E=2048, L=2048] bf16   x[b].T  (e on rows)
  wqk   [E, 1024]        bf16   8 col-blocks: q-heads 0..3, k-heads 0..3,
                                head rows perm'd to (even|odd) order, transposed
  wv    [E, 512]         bf16   v weights, natural order, transposed
  wout  [512, E]         bf16   w_proj[:, g*512:(g+1)*512].T
  cs,ss [128, L]         bf16   rope cos / (-sin|+sin) tables * 128**-0.25
  masks [128, 4*512]     bf16   causal diagonal-block masks
  ones  [128, 128]       fp16   all-ones (softmax denominator broadcast-sum)
Output:
  out   [E, L] fp16  (partial projection, transposed; host adds + transposes
                      in fp32)
"""

from contextlib import ExitStack

import numpy as np
import ml_dtypes

import concourse.bass as bass
import concourse.mybir as mybir
import concourse.tile as tile
from concourse import bacc
from concourse.bass_utils import run_bass_kernel_spmd

BF16 = ml_dtypes.bfloat16
B, L, E, H, HD = 2, 2048, 2048, 16, 128
G = 4            # head groups (cores per batch)
HPG = H // G     # heads per group = 4
NCORES = 8
NE = E // 128    # 16 e-chunks
NLC = L // 512   # 4 l-chunks of 512
NLT = L // 128   # 16 l-tiles of 128
SCALE = float(128.0 ** -0.25)   # per-operand score scale (q and k each)

FP32 = mybir.dt.float32
FP16 = mybir.dt.float16
BF = mybir.dt.bfloat16


def build_nc():
    nc = bacc.Bacc(
        "TRN2",
        target_bir_lowering=False,
        debug=False,
        enable_asserts=False,
        num_devices=NCORES,
    )
    d = {}
    d["xt"] = nc.dram_tensor("xt", [E, L], BF, kind="ExternalInput").ap()
    d["wqk"] = nc.dram_tensor("wqk", [E, 2 * HPG * 128], BF, kind="ExternalInput").ap()
    d["wv"] = nc.dram_tensor("wv", [E, HPG * 128], BF, kind="ExternalInput").ap()
    d["wout"] = nc.dram_tensor("wout", [HPG * 128, E], BF, kind="ExternalInput").ap()
    d["cs"] = nc.dram_tensor("cs", [128, L], BF, kind="ExternalInput").ap()
    d["ss"] = nc.dram_tensor("ss", [128, L], BF, kind="ExternalInput").ap()
    d["masks"] = nc.dram_tensor("masks", [128, 4 * 512], BF,
                                kind="ExternalInput").ap()
    d["ones"] = nc.dram_tensor("ones", [128, 128], mybir.dt.float16,
                               kind="ExternalInput").ap()
    d["out"] = nc.dram_tensor("out", [E, L], mybir.dt.float16,
                              kind="ExternalOutput").ap()

    with tile.TileContext(nc) as tc:
        build_kernel(tc, d)
    nc.compile()
    return nc


def build_kernel(tc, d):
    nc = tc.nc
    EXP = mybir.ActivationFunctionType.Exp

    with ExitStack() as ctx:
        const = ctx.enter_context(tc.tile_pool(name="const", bufs=1))
        qkres = ctx.enter_context(tc.tile_pool(name="qkres", bufs=1))
        vres = ctx.enter_context(tc.tile_pool(name="vres", bufs=1))
        yres = ctx.enter_context(tc.tile_pool(name="yres", bufs=1))
        xs = ctx.enter_context(tc.tile_pool(name="xs", bufs=24))
        atile = ctx.enter_context(tc.tile_pool(name="atile", bufs=8))
        pexp = ctx.enter_context(tc.tile_pool(name="pexp", bufs=10))
        spool = ctx.enter_context(tc.tile_pool(name="spool", bufs=3))
        zpool = ctx.enter_context(tc.tile_pool(name="zpool", bufs=3))
        outst = ctx.enter_context(tc.tile_pool(name="outst", bufs=2))

        # ---- constants / weights ----
        # all startup-critical loads go on the gpsimd (software-descriptor)
        # DMA queue, in PE consumption order; DMAs are emitted inside the
        # phase-1 loop. Tiles allocated here.
        wqk_sb = const.tile([128, NE, 2 * HPG * 128], BF, name="wqk_sb", tag="wqk_sb")
        wqk_r = d["wqk"].rearrange("(ec p) f -> p ec f", p=128)
        cs_sb = const.tile([128, L], BF, name="cs_sb", tag="cs_sb")
        ss_sb = const.tile([128, L], BF, name="ss_sb", tag="ss_sb")
        wv_sb = const.tile([128, NE, HPG * 128], BF, name="wv_sb", tag="wv_sb")
        wv_r = d["wv"].rearrange("(ec p) f -> p ec f", p=128)
        # masks/ones/wout are needed only from the attention phase on; their
        # DMAs are emitted mid-phase-1 (scalar queue) so they don't contend
        # with the startup-critical weight/x loads.
        masks_sb = const.tile([128, 4, 512], BF, name="masks_sb",
                              tag="masks_sb")
        ones_sb = const.tile([128, 128], FP16, name="ones_sb", tag="ones_sb")
        # per-partition bias vector for the biased exp (see phase 2)
        ebias_sb = const.tile([128, 1], FP32, name="ebias_sb", tag="ebias_sb")
        nc.vector.memset(ebias_sb, -9.0)
        wout_sb = const.tile([128, HPG, E], BF, name="wout_sb", tag="wout_sb")

        # ---- residents ----
        q_sb = [qkres.tile([128, L], BF, name=f"q_sb{h}", tag=f"q_sb{h}")
                for h in range(HPG)]
        k_sb = [qkres.tile([128, L], BF, name=f"k_sb{h}", tag=f"k_sb{h}")
                for h in range(HPG)]
        v_sb = vres.tile([128, NLT, HPG * 128], BF, name="v_sb", tag="v_sb")
        y_sb = [yres.tile([128, L], BF, name=f"y_sb{h}", tag=f"y_sb{h}")
                for h in range(HPG)]

        # ================= phase 1: QKV projection + fused rope ============
        # Per-lc PE stream: [lc0: qk-combined | lc>0: qk(q) | qk(k)] | v_pass.
        # The rope half-swap runs entirely off the PE: an SBUF->SBUF DMA on
        # the (otherwise idle) gpsimd queue rotates the q*ss product by 64
        # partitions, then a DVE sub folds it into the destination.
        with tc.tile_pool(name="psum1", bufs=1, space="PSUM") as ps1:

            def acc_tile(nm, tag):
                return ps1.tile([128, 512], FP32, name=nm, tag=tag, bufs=4)

            for lc in range(NLC):
                ls_lo = lc * 512
                cs_lc = cs_sb[:, ls_lo:ls_lo + 512]
                ss_lc = ss_sb[:, ls_lo:ls_lo + 512]

                # x tiles grouped 4 e-chunks per SBUF tile / DMA descriptor
                # (xg[:, j, :] slices act as the per-e tiles)
                xt_r = d["xt"].rearrange("(ec p) l -> p ec l", p=128)
                xt_t = []
                for eg in range(NE // 4):
                    g = xs.tile([128, 4, 512], BF, name=f"xt_{lc}_{eg}",
                                tag="xt", bufs=6)
                    if lc == 0:
                        # lc0 startup: stream per-chunk on sync while each
                        # wqk chunk is split q-half/k-half over the gpsimd/
                        # scalar queues, all in PE consumption order — three
                        # queues share the startup load at fine grain
                        for j in range(4):
                            e = eg * 4 + j
                            nc.sync.dma_start(
                                out=g[:, j, :],
                                in_=xt_r[:, e, ls_lo:ls_lo + 512])
                            nc.gpsimd.dma_start(out=wqk_sb[:, e, 0:512],
                                                in_=wqk_r[:, e, 0:512])
                            nc.scalar.dma_start(out=wqk_sb[:, e, 512:1024],
                                                in_=wqk_r[:, e, 512:1024])
                    else:
                        nc.sync.dma_start(
                            out=g,
                            in_=xt_r[:, eg * 4:eg * 4 + 4,
                                     ls_lo:ls_lo + 512])
                    xt_t.append(g)

                def xt_ap(e, c0=0, c1=512):
                    return xt_t[e // 4][:, e % 4, c0:c1]
                if lc == 0:
                    nc.gpsimd.dma_start(out=cs_sb, in_=d["cs"])
                    nc.gpsimd.dma_start(out=ss_sb, in_=d["ss"])
                    for e in range(NE):
                        nc.gpsimd.dma_start(out=wv_sb[:, e, :],
                                            in_=wv_r[:, e, :])
                if lc == 1:
                    # attention-phase constants: prefetch on the (now idle)
                    # scalar queue, well before first use
                    nc.scalar.dma_start(
                        out=masks_sb,
                        in_=d["masks"].rearrange("p (r f) -> p r f", r=4))
                    nc.scalar.dma_start(out=ones_sb, in_=d["ones"])
                    nc.scalar.dma_start(
                        out=wout_sb,
                        in_=d["wout"].rearrange("(h p) f -> p h f", p=128))

                def qk_pass(halves):
                    # projection matmuls for the given f-block halves
                    # (0 = q heads, 1 = k heads); passing both interleaves
                    # them per e-chunk, which halves the weight-chunk arrival
                    # rate the PE needs (used for the DMA-bound first l-chunk)
                    acc = [acc_tile(f"p{half}_{lc}_{h}",
                                    "pA" if half == 0 else "pB")
                           for half in halves for h in range(HPG)]
                    for e in range(NE):
                        for i, half in enumerate(halves):
                            for h in range(HPG):
                                fb = half * HPG + h
                                nc.tensor.matmul(
                                    acc[i * HPG + h],
                                    lhsT=wqk_sb[:, e, fb * 128:(fb + 1) * 128],
                                    rhs=xt_ap(e),
                                    start=(e == 0), stop=(e == NE - 1))
                    return acc

                def rope_a(acc, which):
                    # a = q*ss (bf16, all 4 heads in one tile so the swap is
                    # 2 DMA descriptors), dst-slice = q*cs ; releases acc
                    a4 = atile.tile([128, HPG, 512], BF,
                                    name=f"a_{which}_{lc}", tag="a", bufs=2)
                    for h in range(HPG):
                        nc.vector.tensor_mul(out=a4[:, h, :], in0=acc[h],
                                             in1=ss_lc)
                        dst = (q_sb if which == "q" else k_sb)[h]
                        nc.vector.tensor_mul(
                            out=dst[:, ls_lo:ls_lo + 512], in0=acc[h], in1=cs_lc)
                    return a4

                def rope_b(a4, which):
                    # dst -= half_swap(a)
                    asw = atile.tile([128, HPG, 512], BF,
                                     name=f"asw_{which}_{lc}",
                                     tag="asw", bufs=2)
                    nc.gpsimd.dma_start(out=asw[0:64, :, :],
                                        in_=a4[64:128, :, :])
                    nc.gpsimd.dma_start(out=asw[64:128, :, :],
                                        in_=a4[0:64, :, :])
                    for h in range(HPG):
                        dst = (q_sb if which == "q" else k_sb)[h]
                        sl = dst[:, ls_lo:ls_lo + 512]
                        nc.vector.tensor_sub(out=sl, in0=sl,
                                             in1=asw[:, h, :])

                def v_pass():
                    # v pass (x tiles stationary -> natural [l, d] layout)
                    for ls in range(4):
                        lt = lc * 4 + ls
                        accv = acc_tile(f"pv_{lt}", "pA")
                        for e in range(NE):
                            nc.tensor.matmul(
                                accv,
                                lhsT=xt_ap(e, ls * 128, (ls + 1) * 128),
                                rhs=wv_sb[:, e, :],
                                start=(e == 0), stop=(e == NE - 1))
                        nc.scalar.copy(out=v_sb[:, lt, :], in_=accv)

                if lc == 0:
                    acc8 = qk_pass((0, 1))
                    accq, acck = acc8[:HPG], acc8[HPG:]
                    aq = rope_a(accq, "q")
                    rope_b(aq, "q")
                    ak = rope_a(acck, "k")
                    rope_b(ak, "k")
                    v_pass()
                else:
                    accq = qk_pass((0,))
                    aq = rope_a(accq, "q")
                    rope_b(aq, "q")
                    acck = qk_pass((1,))
                    ak = rope_a(acck, "k")
                    rope_b(ak, "k")
                    v_pass()

        # ======== phase 2+3: causal attention with interleaved projection ==
        # jobs are ic-major: once all 4 heads finished l-chunk ic, that
        # chunk's output projection is emitted immediately — it fills
        # attention pipeline bubbles and spreads the output DMA.
        #
        # softmax denominator: exp tiles (fp16, exp biased by -9 so the sum
        # can't overflow fp16) are accumulated into an SBUF tile S on the DVE;
        # one ones-matmul per (h, ic) on S reduces+broadcasts Z across
        # partitions — ~8x fewer denominator matmul columns than a per-block
        # ones-matmul.
        with tc.tile_pool(name="psum2", bufs=1, space="PSUM") as ps2:
            # ic order (0,2,3,1): ic0 first makes projection work available
            # after only 16 steps (it fills PE idle in scalar-bound attention
            # stretches), and the last job is a medium (8-block) one so the
            # end-of-kernel finalize chain stays short
            jobs = [(h, ic) for ic in (0, 2, 3, 1) for h in range(HPG)]
            steps = [(ji, jb)
                     for ji, (_h, ic) in enumerate(jobs)
                     for jb in range(4 * ic + 4)]
            LA = 3
            pss_map = {}
            ysum = {}
            ssum = {}

            def emit_s(ji, jb):
                h, ic = jobs[ji]
                # diagonal blocks (r >= 1) have no valid columns below
                # f = 128*r: compute only the valid column range
                r = jb - 4 * ic
                lo = r * 128 if r > 0 else 0
                t = ps2.tile([128, 512], FP32, name=f"pss_{ji}_{jb}",
                             tag="pss", bufs=4)
                nc.tensor.matmul(
                    t[:, lo:],
                    lhsT=k_sb[h][:, jb * 128:(jb + 1) * 128],
                    rhs=q_sb[h][:, ic * 512 + lo:(ic + 1) * 512],
                    start=True, stop=True)
                pss_map[(ji, jb)] = t

            proj_q = []

            def emit_proj_group(tag="pzy", bufs=2):
                lc, ft = proj_q.pop(0)
                po = ps2.tile([128, 512], FP32,
                              name=f"po_{ft}_{lc}", tag=tag,
                              bufs=bufs)
                for hh in range(HPG):
                    nc.tensor.matmul(
                        po,
                        lhsT=wout_sb[:, hh,
                                     ft * 128:(ft + 1) * 128],
                        rhs=y_sb[hh][:, lc * 512:(lc + 1) * 512],
                        start=(hh == 0), stop=(hh == HPG - 1))
                ot = outst.tile([128, 512], mybir.dt.float16,
                                name=f"ot_{ft}_{lc}", tag="ot",
                                bufs=6)
                if ft % 2 == 0:
                    nc.vector.tensor_copy(out=ot, in_=po)
                else:
                    nc.scalar.copy(out=ot, in_=po)
                eng = (nc.sync, nc.gpsimd, nc.sync,
                       nc.scalar)[ft % 4]
                eng.dma_start(
                    out=d["out"][ft * 128:(ft + 1) * 128,
                                 lc * 512:(lc + 1) * 512],
                    in_=ot)

            def make_finalize(ji):
                # two stages, injected into the NEXT job (steps jb==2, 3) so
                # the PE never waits on the exp/S-add/recip chains:
                #   stage 0: psz = colsum(S) (PE reduce+broadcast)
                #   stage 1: zv = 1/psz ; y = psy * zv ; queue projection
                h, ic = jobs[ji]
                psy = ysum.pop(ji)
                S = ssum.pop(ji)
                box = {}

                def st0():
                    # psz briefly borrows a score (pss) bank so the po
                    # rotation keeps both pzy banks
                    psz = ps2.tile([128, 512], FP32, name=f"psz_{ji}",
                                   tag="pss", bufs=4)
                    nc.tensor.matmul(psz, lhsT=ones_sb, rhs=S,
                                     start=True, stop=True)
                    box["psz"] = psz

                def st1():
                    zv = zpool.tile([128, 512], FP32, name=f"zinv_{ji}",
                                    tag="zinv")
                    nc.vector.reciprocal_approx_fast(out=zv, in_=box["psz"])
                    nc.vector.tensor_mul(
                        out=y_sb[h][:, ic * 512:(ic + 1) * 512],
                        in0=psy, in1=zv)
                    if h == HPG - 1:
                        # stage the groups two steps so the y-mul lands first
                        proj_stage.extend((0, (ic, ft)) for ft in range(NE))

                return [st0, st1]

            ptr = 0
            pend = []
            proj_stage = []
            pt_first = {}
            for idx, (ji, jb) in enumerate(steps):
                while ptr < len(steps) and ptr <= idx + LA:
                    emit_s(*steps[ptr])
                    ptr += 1
                if pend and jb in (2, 3):
                    pend[0][jb - 2]()
                    if jb == 3:
                        pend.pop(0)
                h, ic = jobs[ji]
                njb = 4 * ic + 4
                if jb == 0:
                    ysum[ji] = ps2.tile([128, 512], FP32, name=f"psy_{ji}",
                                        tag="psy", bufs=2)
                    ssum[ji] = spool.tile([128, 512], FP16,
                                          name=f"S_{ji}", tag="S")
                psy = ysum[ji]
                S = ssum[ji]
                pss = pss_map.pop((ji, jb))
                r = jb - 4 * ic
                lo = r * 128 if r > 0 else 0
                pt = pexp.tile([128, 512], BF, name=f"pt_{ji}_{jb}",
                               tag="pexp")
                nc.scalar.activation(out=pt[:, lo:], in_=pss[:, lo:],
                                     func=EXP, bias=ebias_sb)
                if r >= 0:
                    # diagonal block: only the first 128 columns of the valid
                    # range hold the per-element triangle; the rest are all-1.
                    # Runs on gpsimd — off the busy DVE/scalar queues.
                    nc.gpsimd.tensor_mul(
                        out=pt[:, lo:lo + 128], in0=pt[:, lo:lo + 128],
                        in1=masks_sb[:, r, lo:lo + 128])
                nc.tensor.matmul(psy[:, lo:],
                                 lhsT=v_sb[:, jb, h * 128:(h + 1) * 128],
                                 rhs=pt[:, lo:],
                                 start=(jb == 0), stop=(jb == njb - 1))
                # S accumulation (fp16, exp pre-scaled by e^-9 so Z fits)
                if jb == 0:
                    pt_first[ji] = pt
                elif jb == 1:
                    p0 = pt_first.pop(ji)
                    nc.vector.tensor_add(out=S[:, lo:], in0=p0[:, lo:],
                                         in1=pt[:, lo:])
                    if lo > 0:
                        nc.vector.tensor_copy(out=S[:, :lo], in_=p0[:, :lo])
                else:
                    nc.vector.tensor_add(out=S[:, lo:], in0=S[:, lo:],
                                         in1=pt[:, lo:])
                if jb == njb - 1:
                    pend.append(make_finalize(ji))
                # spread projection groups between attention steps: one per
                # TWO steps covers the whole phase (64 groups, 160 steps),
                # keeping every stretch PE-bound rather than exp-bound
                if proj_q and idx % 2 == 0:
                    emit_proj_group()
                if proj_stage:
                    lag = proj_stage[0][0]
                    if lag >= 3:
                        proj_q.extend(p[1] for p in proj_stage)
                        proj_stage.clear()
                    else:
                        proj_stage[:] = [(n + 1, p) for n, p in proj_stage]
            for stages in pend:
                for st in stages:
                    st()
            proj_q.extend(p[1] for p in proj_stage)
            while proj_q:
                emit_proj_group()


# ------------------------------------------------------------------ host side

_PERM_IDX = np.concatenate([np.arange(0, 128, 2), np.arange(1, 128, 2)])


def prep_in_maps(x, rope, w_attn, w_proj):
    x = np.asarray(x, np.float32)
    rope = np.asarray(rope, np.float32)
    w_attn = np.asarray(w_attn, np.float32)
    w_proj = np.asarray(w_proj, np.float32)

    sin = rope[:, :, 0]                      # [L, 64]
    cos = rope[:, :, 1]
    cs = (np.concatenate([cos.T, cos.T], 0) * SCALE).astype(BF16)   # [128, L]
    ss = (np.concatenate([-sin.T, sin.T], 0) * SCALE).astype(BF16)

    p = np.arange(128)[:, None]
    f = np.arange(512)[None, :]
    masks = np.zeros((128, 4, 512), np.float32)
    for r in range(4):
        masks[:, r, :] = (r * 128 + p <= f).astype(np.float32)
    masks = masks.reshape(128, 4 * 512).astype(BF16)

    ones = np.ones((128, 128), np.float16)

    xt_b = [np.ascontiguousarray(x[b].T).astype(BF16) for b in range(B)]

    wqk_g, wv_g, wout_g = {}, {}, {}
    for g in range(G):
        heads = [g * HPG + hl for hl in range(HPG)]
        wq = [np.ascontiguousarray(
                 w_attn[h * 128:(h + 1) * 128, :][_PERM_IDX, :].T) for h in heads]
        wk = [np.ascontiguousarray(
                 w_attn[E + h * 128:E + (h + 1) * 128, :][_PERM_IDX, :].T)
              for h in heads]
        wqk_g[g] = np.concatenate(wq + wk, axis=1).astype(BF16)        # [E, 1024]
        wv_g[g] = np.concatenate(
            [w_attn[2 * E + h * 128:2 * E + (h + 1) * 128, :].T for h in heads],
            axis=1).astype(BF16)                                        # [E, 512]
        wout_g[g] = np.ascontiguousarray(
            w_proj[:, g * 512:(g + 1) * 512].T).astype(BF16)            # [512, E]

    in_maps = []
    for c in range(NCORES):
        b, g = divmod(c, G)
        in_maps.append({
            "xt": xt_b[b],
            "wqk": wqk_g[g],
            "wv": wv_g[g],
            "wout": wout_g[g],
            "cs": cs,
            "ss": ss,
            "masks": masks,
            "ones": ones,
        })
    return in_maps


def assemble_output(results):
    out = np.zeros((B, L, E), np.float32)
    for c in range(NCORES):
        b, g = divmod(c, G)
        out[b] += results[c]["out"].T
    return out


_NC = None


def get_nc():
    global _NC
    if _NC is None:
        _NC = build_nc()
    return _NC


def run(x, rope, w_attn, w_proj, trace=False, tmpdir=None):
    nc = get_nc()
    in_maps = prep_in_maps(x, rope, w_attn, w_proj)
    kwargs = {}
    if trace:
        import sys
        import types
        from concourse import bass_utils as _bu
        try:
            from trn_agent_boot.trn_boot import _ntff_profile_via_ctypes
            hook = _ntff_profile_via_ctypes("/opt/axon/libaxon_pjrt.so")
            mod = types.ModuleType("antenv.axon_hooks")
            mod.get_axon_ntff_profile_hook = lambda: hook
            sys.modules["antenv.axon_hooks"] = mod
            _bu.upload_artifacts = lambda dd: dd
        except Exception as e:  # pragma: no cover
            print("trace hook unavailable:", e)
        kwargs = dict(trace=True, tmpdir=tmpdir)
    res = run_bass_kernel_spmd(nc, in_maps, core_ids=list(range(NCORES)), **kwargs)
    return assemble_output(res.results), res


def kernel(x, rope, w_attn, w_proj):
    out, _ = run(x, rope, w_attn, w_proj, trace=False)
    return out



# revision 87
# speedup vs baseline: 1.0113x; 1.0113x over previous
"""Causal self-attention (B=2, L=2048, E=2048, H=16, HD=128) on 8 trn2 cores.

Sharding: core c = (b, g) with b = c // 4 (batch), g = c % 4 (head group of 4).
Each core computes QKV projection for its 4 heads on its batch, causal
attention with RoPE, and a partial output projection (its heads' slice of
w_proj rows). Host sums the 4 partial projections per batch.

All matmuls run in bf16/fp16 with fp32 PSUM accumulation (measured
end-to-end rel. error ~4.3e-3 vs the fp32 reference).

Key device-side structure (per core):
  - phase 1, per 512-wide l-chunk: q/k/v projections as K-accumulated
    matmuls; rope fused right behind each q/k chunk:
        rot = (q * cs) - half_swap(q * ss)      [2 DVE muls + DMA swap + sub]
    with cs/ss host-prebuilt [128, L] tables (softmax scale folded in); the
    half-swap (rope pair partner lives at p +- 64 because head rows are
    perm'd to even|odd order) is an SBUF->SBUF DMA on the gpsimd queue — no
    PE or cross-partition ALU involvement.
  - phase 2: scores computed transposed (sT[j,i] = k_j . q_i) so P@V needs
    no transpose; softmax without max-subtraction: exp biased by -9 so the
    probabilities fit fp16; the denominator is accumulated across key blocks
    on the DVE into an fp16 SBUF tile S, then ONE ones-matmul per (head,
    l-chunk) reduces S over partitions and broadcasts Z (instead of a
    per-block ones-matmul — 8x fewer PE columns); causal masking by skipping
    upper-triangle blocks + 4 static diagonal masks (applied on gpsimd);
    software-pipelined with a 3-deep score-matmul lookahead, and each job's
    finalize (Z-reduce / reciprocal / y-scale) deferred into the next job's
    steps so the PE never waits on the exp/S-add chain.
  - phase 3: partial out-projection, [f, l] layout, fp16 partials; each
    l-chunk's projection is queued when its 4 heads finish and dribbled out
    one 4-matmul group per TWO attention steps — 64 groups cover all 160
    steps, filling PE idle in exp-bound stretches and spreading the output
    DMA. Jobs run in ic order (0,2,3,1) so projection work exists after just
    16 steps and the final finalize chain belongs to a medium-size job.

Scheduling notes (hard-won):
  - The PE runs ~0.42 ns/col in bf16/fp16 only when kept continuously busy;
    it is essentially at the column-count roofline here, so all remaining
    wins come from removing columns or removing stalls.
  - DVE ops cost ~free_size cycles regardless of partition count; 2-byte
    SBUF-only operands run at 2x. GPSIMD tensor ops are ~3x slower and
    cannot touch PSUM; gpsimd's software DMA queue is the fastest loader.
  - The chip sometimes runs whole executions downclocked (2.4 GHz ->
    ~1.8 GHz, +17% wall; matmul median 378ns vs 453ns in the profile),
    mostly after many back-to-back runs (thermal). Benchmark conclusions
    drawn from a hot chip are garbage — rest the device ~90-150s and check
    the matmul-duration median before trusting a comparison.
  - Startup DMAs are split across the sync/gpsimd/scalar queues in exact PE
    consumption order; attention-phase constants prefetch mid-phase-1.

Device layouts (per core):
  xt    [E=2048, L=2048] bf16   x[b].T  (e on rows)
  wqk   [E, 1024]        bf16   8 col-blocks: q-heads 0..3, k-heads 0..3,
                                head rows perm'd to (even|odd) order, transposed
  wv    [E, 512]         bf16   v weights, natural order, transposed
  wout  [512, E]         bf16   w_proj[:, g*512:(g+1)*512].T
  cs,ss [128, L]         bf16   rope cos / (-sin|+sin) tables * 128**-0.25
  masks [128, 4*512]     bf16   causal diagonal-block masks
  ones  [128, 128]       fp16   all-ones (softmax denominator broadcast-sum)
Output:
  out   [E, L] fp16  (partial projection, transposed; host adds + transposes
                      in fp32)
"""

from contextlib import ExitStack

import numpy as np
import ml_dtypes

import concourse.bass as bass
import concourse.mybir as mybir
import concourse.tile as tile
from concourse import bacc
from concourse.bass_utils import run_bass_kernel_spmd

BF16 = ml_dtypes.bfloat16
B, L, E, H, HD = 2, 2048, 2048, 16, 128
G = 4            # head groups (cores per batch)
HPG = H // G     # heads per group = 4
NCORES = 8
NE = E // 128    # 16 e-chunks
NLC = L // 512   # 4 l-chunks of 512
NLT = L // 128   # 16 l-tiles of 128
SCALE = float(128.0 ** -0.25)   # per-operand score scale (q and k each)

FP32 = mybir.dt.float32
FP16 = mybir.dt.float16
BF = mybir.dt.bfloat16


def build_nc():
    nc = bacc.Bacc(
        "TRN2",
        target_bir_lowering=False,
        debug=False,
        enable_asserts=False,
        num_devices=NCORES,
    )
    d = {}
    d["xt"] = nc.dram_tensor("xt", [E, L], BF, kind="ExternalInput").ap()
    d["wqk"] = nc.dram_tensor("wqk", [E, 2 * HPG * 128], BF, kind="ExternalInput").ap()
    d["wv"] = nc.dram_tensor("wv", [E, HPG * 128], BF, kind="ExternalInput").ap()
    d["wout"] = nc.dram_tensor("wout", [HPG * 128, E], BF, kind="ExternalInput").ap()
    d["cs"] = nc.dram_tensor("cs", [128, L], BF, kind="ExternalInput").ap()
    d["ss"] = nc.dram_tensor("ss", [128, L], BF, kind="ExternalInput").ap()
    d["masks"] = nc.dram_tensor("masks", [128, 4 * 512], BF,
                                kind="ExternalInput").ap()
    d["ones"] = nc.dram_tensor("ones", [128, 128], mybir.dt.float16,
                               kind="ExternalInput").ap()
    d["out"] = nc.dram_tensor("out", [E, L], mybir.dt.float16,
                              kind="ExternalOutput").ap()

    with tile.TileContext(nc) as tc:
        build_kernel(tc, d)
    nc.compile()
    return nc


def build_kernel(tc, d):
    nc = tc.nc
    EXP = mybir.ActivationFunctionType.Exp

    with ExitStack() as ctx:
        const = ctx.enter_context(tc.tile_pool(name="const", bufs=1))
        qkres = ctx.enter_context(tc.tile_pool(name="qkres", bufs=1))
        vres = ctx.enter_context(tc.tile_pool(name="vres", bufs=1))
        yres = ctx.enter_context(tc.tile_pool(name="yres", bufs=1))
        xs = ctx.enter_context(tc.tile_pool(name="xs", bufs=24))
        atile = ctx.enter_context(tc.tile_pool(name="atile", bufs=8))
        pexp = ctx.enter_context(tc.tile_pool(name="pexp", bufs=10))
        spool = ctx.enter_context(tc.tile_pool(name="spool", bufs=3))
        zpool = ctx.enter_context(tc.tile_pool(name="zpool", bufs=3))
        outst = ctx.enter_context(tc.tile_pool(name="outst", bufs=2))

        # ---- constants / weights ----
        # all startup-critical loads go on the gpsimd (software-descriptor)
        # DMA queue, in PE consumption order; DMAs are emitted inside the
        # phase-1 loop. Tiles allocated here.
        wqk_sb = const.tile([128, NE, 2 * HPG * 128], BF, name="wqk_sb", tag="wqk_sb")
        wqk_r = d["wqk"].rearrange("(ec p) f -> p ec f", p=128)
        cs_sb = const.tile([128, L], BF, name="cs_sb", tag="cs_sb")
        ss_sb = const.tile([128, L], BF, name="ss_sb", tag="ss_sb")
        wv_sb = const.tile([128, NE, HPG * 128], BF, name="wv_sb", tag="wv_sb")
        wv_r = d["wv"].rearrange("(ec p) f -> p ec f", p=128)
        # masks/ones/wout are needed only from the attention phase on; their
        # DMAs are emitted mid-phase-1 (scalar queue) so they don't contend
        # with the startup-critical weight/x loads.
        masks_sb = const.tile([128, 4, 512], BF, name="masks_sb",
                              tag="masks_sb")
        ones_sb = const.tile([128, 128], FP16, name="ones_sb", tag="ones_sb")
        # per-partition bias vector for the biased exp (see phase 2)
        ebias_sb = const.tile([128, 1], FP32, name="ebias_sb", tag="ebias_sb")
        nc.vector.memset(ebias_sb, -9.0)
        wout_sb = const.tile([128, HPG, E], BF, name="wout_sb", tag="wout_sb")

        # ---- residents ----
        q_sb = [qkres.tile([128, L], BF, name=f"q_sb{h}", tag=f"q_sb{h}")
                for h in range(HPG)]
        k_sb = [qkres.tile([128, L], BF, name=f"k_sb{h}", tag=f"k_sb{h}")
                for h in range(HPG)]
        v_sb = vres.tile([128, NLT, HPG * 128], BF, name="v_sb", tag="v_sb")
        y_sb = [yres.tile([128, L], BF, name=f"y_sb{h}", tag=f"y_sb{h}")
                for h in range(HPG)]

        # ================= phase 1: QKV projection + fused rope ============
        # Per-lc PE stream: [lc0: qk-combined | lc>0: qk(q) | qk(k)] | v_pass.
        # The rope half-swap runs entirely off the PE: an SBUF->SBUF DMA on
        # the (otherwise idle) gpsimd queue rotates the q*ss product by 64
        # partitions, then a DVE sub folds it into the destination.
        with tc.tile_pool(name="psum1", bufs=1, space="PSUM") as ps1:

            def acc_tile(nm, tag):
                return ps1.tile([128, 512], FP32, name=nm, tag=tag, bufs=4)

            for lc in range(NLC):
                ls_lo = lc * 512
                cs_lc = cs_sb[:, ls_lo:ls_lo + 512]
                ss_lc = ss_sb[:, ls_lo:ls_lo + 512]

                # x tiles grouped 4 e-chunks per SBUF tile / DMA descriptor
                # (xg[:, j, :] slices act as the per-e tiles)
                xt_r = d["xt"].rearrange("(ec p) l -> p ec l", p=128)
                xt_t = []
                for eg in range(NE // 4):
                    g = xs.tile([128, 4, 512], BF, name=f"xt_{lc}_{eg}",
                                tag="xt", bufs=6)
                    if lc == 0:
                        # lc0 startup: stream per-chunk on sync while each
                        # wqk chunk is split q-half/k-half over the gpsimd/
                        # scalar queues, all in PE consumption order — three
                        # queues share the startup load at fine grain
                        for j in range(4):
                            e = eg * 4 + j
                            nc.sync.dma_start(
                                out=g[:, j, :],
                                in_=xt_r[:, e, ls_lo:ls_lo + 512])
                            nc.gpsimd.dma_start(out=wqk_sb[:, e, 0:512],
                                                in_=wqk_r[:, e, 0:512])
                            nc.scalar.dma_start(out=wqk_sb[:, e, 512:1024],
                                                in_=wqk_r[:, e, 512:1024])
                    else:
                        nc.sync.dma_start(
                            out=g,
                            in_=xt_r[:, eg * 4:eg * 4 + 4,
                                     ls_lo:ls_lo + 512])
                    xt_t.append(g)

                def xt_ap(e, c0=0, c1=512):
                    return xt_t[e // 4][:, e % 4, c0:c1]
                if lc == 0:
                    nc.gpsimd.dma_start(out=cs_sb, in_=d["cs"])
                    nc.gpsimd.dma_start(out=ss_sb, in_=d["ss"])
                    for e in range(NE):
                        nc.gpsimd.dma_start(out=wv_sb[:, e, :],
                                            in_=wv_r[:, e, :])
                if lc == 1:
                    # attention-phase constants: prefetch on the (now idle)
                    # scalar queue, well before first use
                    nc.scalar.dma_start(
                        out=masks_sb,
                        in_=d["masks"].rearrange("p (r f) -> p r f", r=4))
                    nc.scalar.dma_start(out=ones_sb, in_=d["ones"])
                    nc.scalar.dma_start(
                        out=wout_sb,
                        in_=d["wout"].rearrange("(h p) f -> p h f", p=128))

                def qk_pass(halves):
                    # projection matmuls for the given f-block halves
                    # (0 = q heads, 1 = k heads); passing both interleaves
                    # them per e-chunk, which halves the weight-chunk arrival
                    # rate the PE needs (used for the DMA-bound first l-chunk)
                    acc = [acc_tile(f"p{half}_{lc}_{h}",
                                    "pA" if half == 0 else "pB")
                           for half in halves for h in range(HPG)]
                    for e in range(NE):
                        for i, half in enumerate(halves):
                            for h in range(HPG):
                                fb = half * HPG + h
                                nc.tensor.matmul(
                                    acc[i * HPG + h],
                                    lhsT=wqk_sb[:, e, fb * 128:(fb + 1) * 128],
                                    rhs=xt_ap(e),
                                    start=(e == 0), stop=(e == NE - 1))
                    return acc

                def rope_a(acc, which):
                    # a = q*ss (bf16, all 4 heads in one tile so the swap is
                    # 2 DMA descriptors), dst-slice = q*cs ; releases acc
                    a4 = atile.tile([128, HPG, 512], BF,
                                    name=f"a_{which}_{lc}", tag="a", bufs=2)
                    for h in range(HPG):
                        nc.vector.tensor_mul(out=a4[:, h, :], in0=acc[h],
                                             in1=ss_lc)
                        dst = (q_sb if which == "q" else k_sb)[h]
                        nc.vector.tensor_mul(
                            out=dst[:, ls_lo:ls_lo + 512], in0=acc[h], in1=cs_lc)
                    return a4

                def rope_b(a4, which):
                    # dst -= half_swap(a)
                    asw = atile.tile([128, HPG, 512], BF,
                                     name=f"asw_{which}_{lc}",
                                     tag="asw", bufs=2)
                    nc.gpsimd.dma_start(out=asw[0:64, :, :],
                                        in_=a4[64:128, :, :])
                    nc.gpsimd.dma_start(out=asw[64:128, :, :],
                                        in_=a4[0:64, :, :])
                    for h in range(HPG):
                        dst = (q_sb if which == "q" else k_sb)[h]
                        sl = dst[:, ls_lo:ls_lo + 512]
                        nc.vector.tensor_sub(out=sl, in0=sl,
                                             in1=asw[:, h, :])

                def v_pass():
                    # v pass (x tiles stationary -> natural [l, d] layout)
                    for ls in range(4):
                        lt = lc * 4 + ls
                        accv = acc_tile(f"pv_{lt}", "pA")
                        for e in range(NE):
                            nc.tensor.matmul(
                                accv,
                                lhsT=xt_ap(e, ls * 128, (ls + 1) * 128),
                                rhs=wv_sb[:, e, :],
                                start=(e == 0), stop=(e == NE - 1))
                        nc.scalar.copy(out=v_sb[:, lt, :], in_=accv)

                if lc == 0:
                    acc8 = qk_pass((0, 1))
                    accq, acck = acc8[:HPG], acc8[HPG:]
                    aq = rope_a(accq, "q")
                    rope_b(aq, "q")
                    ak = rope_a(acck, "k")
                    rope_b(ak, "k")
                    v_pass()
                else:
                    accq = qk_pass((0,))
                    aq = rope_a(accq, "q")
                    rope_b(aq, "q")
                    acck = qk_pass((1,))
                    ak = rope_a(acck, "k")
                    rope_b(ak, "k")
                    v_pass()

        # ======== phase 2+3: causal attention with interleaved projection ==
        # jobs are ic-major: once all 4 heads finished l-chunk ic, that
        # chunk's output projection is emitted immediately — it fills
        # attention pipeline bubbles and spreads the output DMA.
        #
        # softmax denominator: exp tiles (fp16, exp biased by -9 so the sum
        # can't overflow fp16) are accumulated into an SBUF tile S on the DVE;
        # one ones-matmul per (h, ic) on S reduces+broadcasts Z across
        # partitions — ~8x fewer denominator matmul columns than a per-block
        # ones-matmul.
        with tc.tile_pool(name="psum2", bufs=1, space="PSUM") as ps2:
            # ic order (0,2,3,1): ic0 first makes projection work available
            # after only 16 steps (it fills PE idle in scalar-bound attention
            # stretches), and the last job is a medium (8-block) one so the
            # end-of-kernel finalize chain stays short
            jobs = [(h, ic) for ic in (0, 2, 3, 1) for h in range(HPG)]
            steps = [(ji, jb)
                     for ji, (_h, ic) in enumerate(jobs)
                     for jb in range(4 * ic + 4)]
            LA = 3
            pss_map = {}
            ysum = {}
            ssum = {}

            def emit_s(ji, jb):
                h, ic = jobs[ji]
                # diagonal blocks (r >= 1) have no valid columns below
                # f = 128*r: compute only the valid column range
                r = jb - 4 * ic
                lo = r * 128 if r > 0 else 0
                t = ps2.tile([128, 512], FP32, name=f"pss_{ji}_{jb}",
                             tag="pss", bufs=4)
                nc.tensor.matmul(
                    t[:, lo:],
                    lhsT=k_sb[h][:, jb * 128:(jb + 1) * 128],
                    rhs=q_sb[h][:, ic * 512 + lo:(ic + 1) * 512],
                    start=True, stop=True)
                pss_map[(ji, jb)] = t

            proj_q = []

            def emit_proj_group(tag="pzy", bufs=2):
                lc, ft = proj_q.pop(0)
                po = ps2.tile([128, 512], FP32,
                              name=f"po_{ft}_{lc}", tag=tag,
                              bufs=bufs)
                for hh in range(HPG):
                    nc.tensor.matmul(
                        po,
                        lhsT=wout_sb[:, hh,
                                     ft * 128:(ft + 1) * 128],
                        rhs=y_sb[hh][:, lc * 512:(lc + 1) * 512],
                        start=(hh == 0), stop=(hh == HPG - 1))
                ot = outst.tile([128, 512], mybir.dt.float16,
                                name=f"ot_{ft}_{lc}", tag="ot",
                                bufs=6)
                if ft % 2 == 0:
                    nc.vector.tensor_copy(out=ot, in_=po)
                else:
                    nc.scalar.copy(out=ot, in_=po)
                eng = (nc.sync, nc.gpsimd, nc.sync,
                       nc.scalar)[ft % 4]
                eng.dma_start(
                    out=d["out"][ft * 128:(ft + 1) * 128,
                                 lc * 512:(lc + 1) * 512],
                    in_=ot)

            def make_finalize(ji):
                # two stages, injected into the NEXT job (steps jb==2, 3) so
                # the PE never waits on the exp/S-add/recip chains:
                #   stage 0: psz = colsum(S) (PE reduce+broadcast)
                #   stage 1: zv = 1/psz ; y = psy * zv ; queue projection
                h, ic = jobs[ji]
                psy = ysum.pop(ji)
                S = ssum.pop(ji)
                box = {}

                def st0():
                    # psz briefly borrows a score (pss) bank so the po
                    # rotation keeps both pzy banks
                    psz = ps2.tile([128, 512], FP32, name=f"psz_{ji}",
                                   tag="pss", bufs=4)
                    nc.tensor.matmul(psz, lhsT=ones_sb, rhs=S,
                                     start=True, stop=True)
                    box["psz"] = psz

                def st1():
                    zv = zpool.tile([128, 512], FP32, name=f"zinv_{ji}",
                                    tag="zinv")
                    nc.vector.reciprocal_approx_fast(out=zv, in_=box["psz"])
                    nc.vector.tensor_mul(
                        out=y_sb[h][:, ic * 512:(ic + 1) * 512],
                        in0=psy, in1=zv)
                    if h == HPG - 1:
                        # stage the groups two steps so the y-mul lands first
                        proj_stage.extend((0, (ic, ft)) for ft in range(NE))

                return [st0, st1]

            ptr = 0
            pend = []
            proj_stage = []
            pt_first = {}
            for idx, (ji, jb) in enumerate(steps):
                while ptr < len(steps) and ptr <= idx + LA:
                    emit_s(*steps[ptr])
                    ptr += 1
                if pend and jb in (2, 3):
                    pend[0][jb - 2]()
                    if jb == 3:
                        pend.pop(0)
                h, ic = jobs[ji]
                njb = 4 * ic + 4
                if jb == 0:
                    ysum[ji] = ps2.tile([128, 512], FP32, name=f"psy_{ji}",
                                        tag="psy", bufs=2)
                    ssum[ji] = spool.tile([128, 512], FP16,
                                          name=f"S_{ji}", tag="S")
                psy = ysum[ji]
                S = ssum[ji]
                pss = pss_map.pop((ji, jb))
                r = jb - 4 * ic
                lo = r * 128 if r > 0 else 0
                pt = pexp.tile([128, 512], BF, name=f"pt_{ji}_{jb}",
                               tag="pexp")
                nc.scalar.activation(out=pt[:, lo:], in_=pss[:, lo:],
                                     func=EXP, bias=ebias_sb)
                if r >= 0:
                    # diagonal block: only the first 128 columns of the valid
                    # range hold the per-element triangle; the rest are all-1.
                    # Runs on gpsimd — off the busy DVE/scalar queues.
                    nc.gpsimd.tensor_mul(
                        out=pt[:, lo:lo + 128], in0=pt[:, lo:lo + 128],
                        in1=masks_sb[:, r, lo:lo + 128])
                nc.tensor.matmul(psy[:, lo:],
                                 lhsT=v_sb[:, jb, h * 128:(h + 1) * 128],
                                 rhs=pt[:, lo:],
                                 start=(jb == 0), stop=(jb == njb - 1))
                # S accumulation (fp16, exp pre-scaled by e^-9 so Z fits)
                if jb == 0:
                    pt_first[ji] = pt
                elif jb == 1:
                    p0 = pt_first.pop(ji)
                    nc.vector.tensor_add(out=S[:, lo:], in0=p0[:, lo:],
                                         in1=pt[:, lo:])
                    if lo > 0:
                        nc.vector.tensor_copy(out=S[:, :lo], in_=p0[:, :lo])
                else:
                    nc.vector.tensor_add(out=S[:, lo:], in0=S[:, lo:],
                                         in1=pt[:, lo:])
                if jb == njb - 1:
                    pend.append(make_finalize(ji))
                # spread projection groups between attention steps: one per
                # TWO steps covers the whole phase (64 groups, 160 steps),
                # keeping every stretch PE-bound rather than exp-bound
                if proj_q and idx % 2 == 0:
                    emit_proj_group()
                if proj_stage:
                    lag = proj_stage[0][0]
                    if lag >= 2:
                        proj_q.extend(p[1] for p in proj_stage)
                        proj_stage.clear()
                    else:
                        proj_stage[:] = [(n + 1, p) for n, p in proj_stage]
            for stages in pend:
                for st in stages:
                    st()
            proj_q.extend(p[1] for p in proj_stage)
            # at the flush all score banks are dead: rotate po through them
            # for a deeper pipeline
            while proj_q:
                emit_proj_group(tag="pss", bufs=4)


# ------------------------------------------------------------------ host side

_PERM_IDX = np.concatenate([np.arange(0, 128, 2), np.arange(1, 128, 2)])


def prep_in_maps(x, rope, w_attn, w_proj):
    x = np.asarray(x, np.float32)
    rope = np.asarray(rope, np.float32)
    w_attn = np.asarray(w_attn, np.float32)
    w_proj = np.asarray(w_proj, np.float32)

    sin = rope[:, :, 0]                      # [L, 64]
    cos = rope[:, :, 1]
    cs = (np.concatenate([cos.T, cos.T], 0) * SCALE).astype(BF16)   # [128, L]
    ss = (np.concatenate([-sin.T, sin.T], 0) * SCALE).astype(BF16)

    p = np.arange(128)[:, None]
    f = np.arange(512)[None, :]
    masks = np.zeros((128, 4, 512), np.float32)
    for r in range(4):
        masks[:, r, :] = (r * 128 + p <= f).astype(np.float32)
    masks = masks.reshape(128, 4 * 512).astype(BF16)

    ones = np.ones((128, 128), np.float16)

    xt_b = [np.ascontiguousarray(x[b].T).astype(BF16) for b in range(B)]

    wqk_g, wv_g, wout_g = {}, {}, {}
    for g in range(G):
        heads = [g * HPG + hl for hl in range(HPG)]
        wq = [np.ascontiguousarray(
                 w_attn[h * 128:(h + 1) * 128, :][_PERM_IDX, :].T) for h in heads]
        wk = [np.ascontiguousarray(
                 w_attn[E + h * 128:E + (h + 1) * 128, :][_PERM_IDX, :].T)
              for h in heads]
        wqk_g[g] = np.concatenate(wq + wk, axis=1).astype(BF16)        # [E, 1024]
        wv_g[g] = np.concatenate(
            [w_attn[2 * E + h * 128:2 * E + (h + 1) * 128, :].T for h in heads],
            axis=1).astype(BF16)                                        # [E, 512]
        wout_g[g] = np.ascontiguousarray(
            w_proj[:, g * 512:(g + 1) * 512].T).astype(BF16)            # [512, E]

    in_maps = []
    for c in range(NCORES):
        b, g = divmod(c, G)
        in_maps.append({
            "xt": xt_b[b],
            "wqk": wqk_g[g],
            "wv": wv_g[g],
            "wout": wout_g[g],
            "cs": cs,
            "ss": ss,
            "masks": masks,
            "ones": ones,
        })
    return in_maps


def assemble_output(results):
    out = np.zeros((B, L, E), np.float32)
    for c in range(NCORES):
        b, g = divmod(c, G)
        out[b] += results[c]["out"].T
    return out


_NC = None


def get_nc():
    global _NC
    if _NC is None:
        _NC = build_nc()
    return _NC


def run(x, rope, w_attn, w_proj, trace=False, tmpdir=None):
    nc = get_nc()
    in_maps = prep_in_maps(x, rope, w_attn, w_proj)
    kwargs = {}
    if trace:
        import sys
        import types
        from concourse import bass_utils as _bu
        try:
            from trn_agent_boot.trn_boot import _ntff_profile_via_ctypes
            hook = _ntff_profile_via_ctypes("/opt/axon/libaxon_pjrt.so")
            mod = types.ModuleType("antenv.axon_hooks")
            mod.get_axon_ntff_profile_hook = lambda: hook
            sys.modules["antenv.axon_hooks"] = mod
            _bu.upload_artifacts = lambda dd: dd
        except Exception as e:  # pragma: no cover
            print("trace hook unavailable:", e)
        kwargs = dict(trace=True, tmpdir=tmpdir)
    res = run_bass_kernel_spmd(nc, in_maps, core_ids=list(range(NCORES)), **kwargs)
    return assemble_output(res.results), res


def kernel(x, rope, w_attn, w_proj):
    out, _ = run(x, rope, w_attn, w_proj, trace=False)
    return out

